# revision 1
# baseline (speedup 1.0000x reference)
"""Trainium2 Bass kernel for nn_LogicConv3d (differentiable-logic conv tree).

Problem (hardcoded): x [16,64,32,32] f32; idx_a/idx_b [64,900,64,3] i32;
w0..w6 [s,64,16] f32 (s = 64,32,16,8,4,2,1). Output [16,64,900,1] f32.

Math: per (kernel k, window p): gather 64 (a,b) leaf pairs from x, blend each
pair with soft-gate coefficients (softmax(w) @ GATE_M), then 6 more pairwise
tree levels.  mix(a,b) = c0 + c1*a + c2*b + c3*a*b.

Mapping:
 - F-sharding: core i handles batches (2i, 2i+1); all 64 kernels on every core
   -> the device program and all tables are identical across cores (pure SPMD);
   only the x-slice differs.
 - Indices are separable: idx[k,p,s] = (ha+hh_p, wa+ww_p, ca) so the leaf
   gather per (k,s) is a 30x30 crop of image x[b, ca] at (ha, wa).  The two
   batches are interleaved host-side (xsrc[c,h,w,b]) so ONE 1920-float
   consecutive run starting at (ca*1024 + ha*32 + wa)*2 contains the whole
   crop for both batches at positions 64*r + 2*q + b (q<30).  One
   indirect-DMA index per SBUF partition row fetches it at full bandwidth.
 - Tree levels run on-chip: partition dim = (node-msb, kernel), free =
   (window, batch).  Tiles at each level are keyed by the low bits of the
   node index so every merge op reads two full tiles at equal partition
   bases (HW constraint).
 - mix is 3 ops: ACT: p = c3*a + c2'; DVE stt: q = (b + beta) * p;
   DVE stt: r = (a * c1) + q; the additive constant c0 (+c1*alpha) folds into
   the next level's scalars (tree edges are single-use), added once at the end.
"""
import numpy as np

B, C, H, W = 16, 64, 32, 32
K = 64
RF = 3
DEPTH = 6
S = 64
PW = 30            # windows per axis
P = PW * PW        # 900
NCORES = 8
B2 = 2             # batches per core
F = P * B2         # free size (windows x batches)
XPAD = 131088      # 2*C*H*W + 16 pad (gather tail can run 4 past the end)

GATE_M = np.array([
    [0, 0, 0, 0], [0, 0, 0, 1], [0, 1, 0, -1], [0, 1, 0, 0],
    [0, 0, 1, -1], [0, 0, 1, 0], [0, 1, 1, -2], [0, 1, 1, -1],
    [1, -1, -1, 1], [1, -1, -1, 2], [1, 0, -1, 0], [1, 0, -1, 1],
    [1, -1, 0, 0], [1, -1, 0, 1], [1, 0, 0, -1], [1, 0, 0, 0],
], dtype=np.float32)  # [16 gates, 4] -> c0,c1,c2,c3 = GATE_M.T @ softmax(w)


# ---------------------------------------------------------------------------
# static schedule: the merge-tree op list
# ---------------------------------------------------------------------------
def _build_schedule():
    """Each mix op: dict(level, key, lanes, base, node[lanes], kern[lanes]).
    L0 ops read gather tiles A_key/B_key; level l>=1 ops read T_{l-1}[2k],[2k+1].
    DFS order keeps the live tile set small."""
    ops = []

    def emit(l, key):
        if l == 0:
            lanes = np.arange(128)
            ops.append(dict(level=0, key=key, lanes=128, base=0,
                            node=key + 32 * (lanes >> 6), kern=lanes & 63))
            return
        emit(l - 1, 2 * key)
        emit(l - 1, 2 * key + 1)
        lanes = np.arange(128)
        nbits_out = 6 - l
        ops.append(dict(level=l, key=key, lanes=128, base=0,
                        node=((lanes >> 6) << (nbits_out - 1)) + key,
                        kern=lanes & 63))

    emit(4, 0)
    emit(4, 1)
    # L5: one full op; node i5 = lane>>6 (a DMA then realigns the top half
    # to a base-0 tile for L6's equal-base inputs)
    lanes = np.arange(128)
    ops.append(dict(level=5, key=0, lanes=128, base=0,
                    node=lanes >> 6, kern=lanes & 63))
    lanes = np.arange(64)
    ops.append(dict(level=6, key=0, lanes=64, base=0,
                    node=np.zeros(64, np.int64), kern=lanes))
    return ops


_SCHED = _build_schedule()
_NMIX = len(_SCHED)          # 65
_NCOLS = 4 * _NMIX + 4       # + final gamma column block


def _softmax_f32(w):
    w = w.astype(np.float64)
    m = w.max(-1, keepdims=True)
    e = np.exp(w - m)
    return e / e.sum(-1, keepdims=True)


def _coef_tables(ws):
    """ws = [w0..w6]. Returns coef matrix [128, _NCOLS] f32 with per-op scalar
    columns (c3, bias, beta, c1) and the final gamma column."""
    cs = []
    for wl in ws:
        p = _softmax_f32(wl)                      # [s, K, 16] f64
        cs.append(np.einsum('skg,gj->skj', p, GATE_M.astype(np.float64)))
    gamma = [None] * 7
    gamma[0] = cs[0][:, :, 0]                     # c0, alpha=0 at leaves
    for l in range(1, 7):
        gamma[l] = cs[l][:, :, 0] + cs[l][:, :, 1] * gamma[l - 1][0::2]
    coef = np.zeros((128, _NCOLS), dtype=np.float64)
    for i, op in enumerate(_SCHED):
        l, node, kern = op['level'], op['node'], op['kern']
        rows = op['base'] + np.arange(op['lanes'])
        c = cs[l][node, kern]                     # [lanes, 4]
        if l == 0:
            alpha = np.zeros(op['lanes'])
            beta = np.zeros(op['lanes'])
        else:
            alpha = gamma[l - 1][2 * node, kern]
            beta = gamma[l - 1][2 * node + 1, kern]
        coef[rows, 4 * i + 0] = c[:, 3]                      # ACT scale = c3
        coef[rows, 4 * i + 1] = c[:, 2] + alpha * c[:, 3]    # ACT bias
        coef[rows, 4 * i + 2] = beta                         # stt1 scalar
        coef[rows, 4 * i + 3] = c[:, 1]                      # stt2 scalar = c1
    coef[0:64, 4 * _NMIX] = gamma[6][0, :]                   # final add
    return coef.astype(np.float32)


def _offset_tables(idx_a, idx_b):
    """Gather index tables [128, 64] i32: col = 2*t + side.
    Element offsets into the b-interleaved x-slice."""
    offs = np.zeros((128, 64), dtype=np.int64)
    for op in _SCHED:
        if op['level'] != 0:
            continue
        t = op['key']
        for side, idx in ((0, idx_a), (1, idx_b)):
            ha = idx[op['kern'], 0, op['node'], 0].astype(np.int64)
            wa = idx[op['kern'], 0, op['node'], 1].astype(np.int64)
            ca = idx[op['kern'], 0, op['node'], 2].astype(np.int64)
            offs[:, 2 * t + side] = (ca * (H * W) + ha * W + wa) * B2
    return offs.astype(np.int32)


# ---------------------------------------------------------------------------
# numpy emulator (mirrors the device schedule exactly; for validation)
# ---------------------------------------------------------------------------
def _emulate_core(xp, offs, coef):
    """xp: [XPAD] f32 b-interleaved slice. Returns [64, F] f32 (hh,ww,b)."""
    tiles = {}
    for i, op in enumerate(_SCHED):
        l, key, n, base = op['level'], op['key'], op['lanes'], op['base']
        rows = base + np.arange(n)
        sc = coef[rows, 4 * i + 0][:, None]
        bi = coef[rows, 4 * i + 1][:, None]
        be = coef[rows, 4 * i + 2][:, None]
        c1 = coef[rows, 4 * i + 3][:, None]
        if l == 0:
            ab = []
            for side in (0, 1):
                o = offs[:, 2 * key + side]
                raw = xp[o[:, None] + np.arange(1920)[None, :]]
                ab.append(raw.reshape(128, 30, 32, 2)[:, :, :30, :]
                          .reshape(128, F))
            a, b = ab
        elif l < 5:
            a = tiles[(l - 1, 2 * key)]
            b = tiles[(l - 1, 2 * key + 1)]
        elif l == 5:
            a = tiles[(4, 0)]
            b = tiles[(4, 1)]
        else:
            a = tiles['T5'][0:64]
            b = tiles['T5'][64:128]
        p = a * sc + bi
        q = (b + be) * p
        r = a * c1 + q
        if l == 5:
            tiles['T5'] = r
        else:
            tiles[(l, key)] = r
    return tiles[(6, 0)]


# ---------------------------------------------------------------------------
# Bass program (built once, cached)
# ---------------------------------------------------------------------------
_BASS_CACHE = {}


def _build_bass():
    if 'nc' in _BASS_CACHE:
        return _BASS_CACHE['nc']
    import concourse.bass as bass
    import concourse.mybir as mybir
    import concourse.tile as tile
    import concourse.bacc as bacc

    f32 = mybir.dt.float32
    nc = bacc.Bacc("TRN2", target_bir_lowering=False, debug=False,
                   num_devices=NCORES)
    xsrc_d = nc.dram_tensor("xsrc", [XPAD, 1], f32, kind="ExternalInput").ap()
    offs_d = nc.dram_tensor("offs", [128, 64], mybir.dt.int32,
                            kind="ExternalInput").ap()
    coef_d = nc.dram_tensor("coef", [128, _NCOLS], f32,
                            kind="ExternalInput").ap()
    out_d = nc.dram_tensor("out", [64, F], f32, kind="ExternalOutput").ap()

    AL = mybir.AluOpType
    ACTF = mybir.ActivationFunctionType

    def raw_view(t):      # [128,1920] -> [128,30,30,2] strided (skip w=30,31)
        return t[:].rearrange("p (h w b) -> p h w b",
                              h=30, w=32, b=2)[:, :, 0:30, :]

    def shp(x):           # compact [n,1800] AP -> [n,30,30,2]
        return x.rearrange("p (h w b) -> p h w b", h=30, w=30, b=2)

    with tile.TileContext(nc) as tc:
        with (
            tc.tile_pool(name="const", bufs=1) as pc,
            tc.tile_pool(name="ab", bufs=4) as pab,
            tc.tile_pool(name="lvl", bufs=2) as plv,
            tc.tile_pool(name="t0p", bufs=2) as pt0,
            tc.tile_pool(name="tmp", bufs=6) as ptmp,
            tc.tile_pool(name="fin", bufs=1) as pfin,
        ):
            offs_t = pc.tile([128, 64], mybir.dt.int32, tag="offs",
                             name="offs_t")
            nc.gpsimd.dma_start(offs_t[:], offs_d[:])
            coef_t = pc.tile([128, _NCOLS], f32, tag="coef", name="coef_t")
            nc.sync.dma_start(coef_t[:], coef_d[:])
            warm_t = pc.tile([1, 8], f32, tag="warm", name="warm_t")
            nc.scalar.activation(warm_t[:], coef_t[0:1, 0:8],
                                 ACTF.Identity, bias=0.0, scale=1.0)

            tiles = {}
            for i, op in enumerate(_SCHED):
                l, key, n, base = op['level'], op['key'], op['lanes'], op['base']
                sl = slice(base, base + n)
                sc = coef_t[sl, 4 * i + 0:4 * i + 1]
                bi = coef_t[sl, 4 * i + 1:4 * i + 2]
                be = coef_t[sl, 4 * i + 2:4 * i + 3]
                c1 = coef_t[sl, 4 * i + 3:4 * i + 4]
                if l == 0 and key == 0:
                    # first triple: gather + compute in h-halves so the DVE
                    # stream starts ~4us earlier (smaller first transfers)
                    r_t = pt0.tile([128, F], f32, tag="T0", name="t0_0")
                    tiles[(0, 0)] = r_t
                    for half in (0, 1):
                        ah = pab.tile([128, 960], f32, tag="A", name="at")
                        bh = pab.tile([128, 960], f32, tag="B", name="bt")
                        for side, dst in ((0, ah), (1, bh)):
                            nc.gpsimd.indirect_dma_start(
                                out=dst[:], out_offset=None, in_=xsrc_d[:],
                                in_offset=bass.IndirectOffsetOnAxis(
                                    ap=offs_t[:, side:side + 1], axis=0),
                                element_offset=960 * half)
                        av = ah[:].rearrange("p (h w b) -> p h w b",
                                             h=15, w=32, b=2)[:, :, 0:30, :]
                        bv = bh[:].rearrange("p (h w b) -> p h w b",
                                             h=15, w=32, b=2)[:, :, 0:30, :]
                        ph = ptmp.tile([128, F], f32, tag="p", name="p")
                        phv = ph[:, 0:900].rearrange(
                            "p (h w b) -> p h w b", h=15, w=30, b=2)
                        nc.scalar.activation(phv, av, ACTF.Identity,
                                             bias=bi, scale=sc)
                        nc.vector.scalar_tensor_tensor(
                            out=phv, in0=bv, scalar=be, in1=phv,
                            op0=AL.add, op1=AL.mult)
                        rhv = shp(r_t[:])[:, 15 * half:15 * half + 15, :, :]
                        nc.vector.scalar_tensor_tensor(
                            out=rhv, in0=av, scalar=c1, in1=phv,
                            op0=AL.mult, op1=AL.add)
                    continue
                if l == 0:
                    at = pab.tile([128, 1920], f32, tag="A", name="at")
                    bt = pab.tile([128, 1920], f32, tag="B", name="bt")
                    for side, dst in ((0, at), (1, bt)):
                        nc.gpsimd.indirect_dma_start(
                            out=dst[:], out_offset=None, in_=xsrc_d[:],
                            in_offset=bass.IndirectOffsetOnAxis(
                                ap=offs_t[:, 2 * key + side:
                                          2 * key + side + 1], axis=0))
                    a_ap, b_ap = raw_view(at), raw_view(bt)
                elif l < 5:
                    a_ap = shp(tiles[(l - 1, 2 * key)][:])
                    b_ap = shp(tiles[(l - 1, 2 * key + 1)][:])
                elif l == 5:
                    a_ap = shp(tiles[(4, 0)][:])
                    b_ap = shp(tiles[(4, 1)][:])
                else:
                    a_ap = shp(tiles['T5'][0:64, :])
                    b_ap = shp(tiles['T5b'][:])

                if base != 0:
                    p_full = ptmp.tile([128, F], f32, tag="p", name="p")
                    p_ap = shp(p_full[sl, :])
                else:
                    p_t = ptmp.tile([n, F], f32, tag="p", name="p")
                    p_ap = shp(p_t[:])
                q_ap = p_ap  # in-place: q overwrites p
                nc.scalar.activation(p_ap, a_ap, ACTF.Identity,
                                     bias=bi, scale=sc)
                nc.vector.scalar_tensor_tensor(
                    out=q_ap, in0=b_ap, scalar=be, in1=p_ap,
                    op0=AL.add, op1=AL.mult)
                if l == 5:
                    r_t = pfin.tile([128, F], f32, tag="T5", name="t5")
                    tiles['T5'] = r_t
                    r_ap = shp(r_t[:])
                elif l == 6:
                    # compute + store output in h-halves so the DMA of half 0
                    # overlaps the stt of half 1; final gamma add happens on host
                    r_t = pfin.tile([64, F], f32, tag="T6", name="t6")
                    for hh in (0, 1):
                        hs = (slice(None), slice(15 * hh, 15 * hh + 15),
                              slice(None), slice(None))
                        nc.vector.scalar_tensor_tensor(
                            out=shp(r_t[:])[hs], in0=a_ap[hs], scalar=c1,
                            in1=q_ap[hs], op0=AL.mult, op1=AL.add)
                        nc.sync.dma_start(
                            out_d[:, 900 * hh:900 * hh + 900],
                            r_t[:, 900 * hh:900 * hh + 900])
                    continue_l6 = True
                    r_ap = None
                else:
                    pool = pt0 if l == 0 else plv
                    r_t = pool.tile([128, F], f32, tag=f"T{l}",
                                    name=f"t{l}_{key}")
                    tiles[(l, key)] = r_t
                    r_ap = shp(r_t[:])
                if r_ap is not None:
                    nc.vector.scalar_tensor_tensor(
                        out=r_ap, in0=a_ap, scalar=c1, in1=q_ap,
                        op0=AL.mult, op1=AL.add)
                if l == 5:
                    t5b = pfin.tile([64, F], f32, tag="T5b", name="t5b")
                    tiles['T5b'] = t5b
                    nc.sync.dma_start(t5b[:], r_t[64:128, :])
    nc.compile()
    _BASS_CACHE['nc'] = nc
    return nc


def _prep_inputs(x, idx_a, idx_b, ws):
    coef = _coef_tables(ws)
    offs = _offset_tables(idx_a, idx_b)
    x = np.ascontiguousarray(x, dtype=np.float32)
    in_maps = []
    for core in range(NCORES):
        # b-interleaved slice: [C,H,W,B2]
        xs = x[B2 * core:B2 * core + B2].transpose(1, 2, 3, 0)
        xp = np.zeros((XPAD,), dtype=np.float32)
        xp[:B2 * C * H * W] = xs.reshape(-1)
        in_maps.append({"xsrc": xp.reshape(XPAD, 1), "offs": offs,
                        "coef": coef})
    return in_maps


def _assemble(core_outs, gamma):
    """core_outs: list of [64, F=(hh,ww,b)]; gamma [64] -> [16,64,900,1]."""
    full = np.stack(core_outs).astype(np.float32)   # [8, 64, 1800]
    full = full + gamma.astype(np.float32)[None, :, None]
    full = full.reshape(NCORES, K, P, B2)           # [core, k, p, b_local]
    full = full.transpose(0, 3, 1, 2).reshape(B, K, P, 1)
    return np.ascontiguousarray(full.astype(np.float32))


def kernel(x, idx_a, idx_b, w0, w1, w2, w3, w4, w5, w6):
    ws = [np.asarray(w, dtype=np.float32) for w in
          (w0, w1, w2, w3, w4, w5, w6)]
    x = np.asarray(x, dtype=np.float32)
    idx_a = np.asarray(idx_a, dtype=np.int32)
    idx_b = np.asarray(idx_b, dtype=np.int32)
    in_maps = _prep_inputs(x, idx_a, idx_b, ws)
    nc = _build_bass()
    from concourse.bass_utils import run_bass_kernel_spmd
    res = run_bass_kernel_spmd(nc, in_maps, core_ids=list(range(NCORES)))
    gamma = in_maps[0]["coef"][0:64, 4 * _NMIX]
    return _assemble([r["out"] for r in res.results], gamma)


def kernel_emulate(x, idx_a, idx_b, w0, w1, w2, w3, w4, w5, w6):
    """Pure-numpy emulation of the exact device schedule (debug aid)."""
    ws = [np.asarray(w, dtype=np.float32) for w in
          (w0, w1, w2, w3, w4, w5, w6)]
    in_maps = _prep_inputs(np.asarray(x, np.float32),
                           np.asarray(idx_a, np.int32),
                           np.asarray(idx_b, np.int32), ws)
    outs = [_emulate_core(m["xsrc"].reshape(-1), m["offs"], m["coef"])
            for m in in_maps]
    return _assemble(outs, in_maps[0]["coef"][0:64, 4 * _NMIX])



# revision 2
# speedup vs baseline: 1.3235x; 1.3235x over previous
"""Trainium2 Bass kernel for nn_LogicConv3d (differentiable-logic conv tree).

Problem (hardcoded): x [16,64,32,32] f32; idx_a/idx_b [64,900,64,3] i32;
w0..w6 [s,64,16] f32 (s = 64,32,16,8,4,2,1). Output [16,64,900,1] f32.

v2 design (fp16, product-form tree, batched dma_gather):
 - Sharding: core i handles batches (2i, 2i+1), all 64 kernels (pure SPMD).
 - Algebra: every stored node value is an affine image X = (V - t)/s of the
   true node value V in [0,1]. Choosing the children's t as -C2/C3, -C1/C3
   makes each tree node an exact PRODUCT of its children's stored values,
   so a node costs 1 tensor_tensor(mult) + 1 tensor_scalar (affine fix) --
   both have DVE fast modes in fp16 (unlike scalar_tensor_tensor, which has
   none).  L0 additionally pre-shifts the raw gathered leaves (2 ts ops).
   All constants fold into the scalars; no final host-side gamma.
 - fp16 end-to-end halves both gather DMA bytes and DVE cycle counts
   (ts: 4x mode, tt: 2x mode).  Validated numerically: rel err ~7e-4 vs
   the 2e-2 gate.
 - Gather: x-slice is laid out in HBM as 9 shifted copies ([576, 2048] fp16,
   row (dh*3+dw)*64+c = channel c image shifted by (dh,dw), b-interleaved)
   so leaf crops become aligned rows; one dma_gather instruction fetches 4
   tile-sides (512 rows x 1920 elems) -> 16 SWDGE dispatches total instead
   of 64+, freeing the Pool engine for compute offload.
 - Engine balance: ops are assigned to DVE / Act / Pool by a greedy
   makespan balancer using the per-engine cost model.
 - Compute is done "wide" (1920 = 30h x 32w x 2b free elems per lane,
   garbage at w=30,31) so every operand is packed (fast-mode eligible);
   the final activation compacts to 900 windows x 2 batches in fp32.
"""
import numpy as np

B, C, H, W = 16, 64, 32, 32
K = 64
DEPTH = 6
PW = 30
P = PW * PW        # 900
NCORES = 8
B2 = 2             # batches per core
FW = 1920          # wide free: 30h * 32w * 2b
NSH = 9            # (dh,dw) shifts
ROWS = NSH * C     # 576 gather source rows
RSTEP = 2048       # gather row stride (elements)
GB = 4             # tile-sides per dma_gather
NG = 64 // GB      # 16 gather instructions
NIDX = GB * 128    # idxs per gather

GATE_M = np.array([
    [0, 0, 0, 0], [0, 0, 0, 1], [0, 1, 0, -1], [0, 1, 0, 0],
    [0, 0, 1, -1], [0, 0, 1, 0], [0, 1, 1, -2], [0, 1, 1, -1],
    [1, -1, -1, 1], [1, -1, -1, 2], [1, 0, -1, 0], [1, 0, -1, 1],
    [1, -1, 0, 0], [1, -1, 0, 1], [1, 0, 0, -1], [1, 0, 0, 0],
], dtype=np.float64)


def _softmax64(w):
    w = np.asarray(w, np.float64)
    e = np.exp(w - w.max(-1, keepdims=True))
    return e / e.sum(-1, keepdims=True)


# ---------------------------------------------------------------------------
# static op schedule with greedy engine assignment
# ---------------------------------------------------------------------------
def _schedule():
    """DFS op list.  Engines: 'v' DVE, 'a' Act, 'p' Pool (gpsimd)."""
    ops = []
    col = [0]
    loads = {'v': 0.0, 'a': 0.0, 'p': NG * 1.6}   # Pool pre-loaded: dispatch
    TS = {'v': 0.60, 'a': 1.70, 'p': 2.77}
    TT = {'v': 1.05, 'p': 3.91}
    ngather = [0]

    def pick(cost):
        e = min(cost, key=lambda k: loads[k] + cost[k])
        loads[e] += cost[e]
        return e

    def need_gather(upto):
        while ngather[0] <= min(upto, NG - 1):
            ops.append({'kind': 'gather', 'g': ngather[0]})
            ngather[0] += 1

    def alloc_col():
        c = col[0]
        col[0] += 2
        return c

    def emit(l, key):
        if l == 0:
            t = key
            need_gather(t // 2 + 2)
            ops.append({'kind': 'ts_side', 'side': 0, 't': t,
                        'col': alloc_col(), 'eng': pick(TS)})
            ops.append({'kind': 'ts_side', 'side': 1, 't': t,
                        'col': alloc_col(), 'eng': pick(TS)})
            ops.append({'kind': 'tt0', 't': t, 'eng': pick(TT)})
            ops.append({'kind': 'fix', 'l': 0, 'key': t,
                        'col': alloc_col(), 'eng': pick(TS)})
            return
        emit(l - 1, 2 * key)
        emit(l - 1, 2 * key + 1)
        ops.append({'kind': 'tt', 'l': l, 'key': key, 'eng': pick(TT)})
        if l < 6:
            ops.append({'kind': 'fix', 'l': l, 'key': key,
                        'col': alloc_col(), 'eng': pick(TS)})
    emit(5, 0)
    ops.append({'kind': 'l6', 'col': alloc_col()})
    return ops, col[0]


_OPS, _NCOL = _schedule()

_LANES = np.arange(128)
_HI = _LANES >> 6
_KERN = _LANES & 63


def _node_of(l, key):
    """Tree-node index per lane for a level-l tile with the given key."""
    if l == 0:
        return key + 32 * _HI
    return (_HI << (5 - l)) + key


# ---------------------------------------------------------------------------
# host tables
# ---------------------------------------------------------------------------
def _build_tables(ws):
    """Per-(node,kern) scalars, f64.  Returns (l0ab, p2):
    l0ab = (qa1,qa2,qb1,qb2) [64,K] leaf-side ts scalars;
    p2[l] = (alpha,beta) [S_l,K] fix-ts scalars at level l."""
    cs = [np.einsum('skg,gj->skj', _softmax64(w), GATE_M) for w in ws]
    s_req = [np.ones((2 ** (DEPTH - l), K)) for l in range(DEPTH + 1)]
    t_req = [np.zeros((2 ** (DEPTH - l), K)) for l in range(DEPTH + 1)]
    for l in range(DEPTH, 0, -1):
        c = cs[l]
        c1, c2, c3 = c[..., 1], c[..., 2], c[..., 3]
        tA, tB = -c2 / c3, -c1 / c3
        t_req[l - 1][0::2], t_req[l - 1][1::2] = tA, tB
        s_req[l - 1][0::2] = (1 + np.abs(tA)) / 2
        s_req[l - 1][1::2] = (1 + np.abs(tB)) / 2
    c = cs[0]
    c1, c2, c3 = c[..., 1], c[..., 2], c[..., 3]
    ta, tb = -c2 / c3, -c1 / c3
    sa, sb = (1 + np.abs(ta)) / 2, (1 + np.abs(tb)) / 2
    l0ab = (1 / sa, -ta / sa, 1 / sb, -tb / sb)
    p2 = []
    for l in range(DEPTH + 1):
        c = cs[l]
        c0, c1, c2, c3 = c[..., 0], c[..., 1], c[..., 2], c[..., 3]
        if l == 0:
            tAc, tBc, sA, sB = ta, tb, sa, sb
        else:
            tAc, tBc = t_req[l - 1][0::2], t_req[l - 1][1::2]
            sA, sB = s_req[l - 1][0::2], s_req[l - 1][1::2]
        D0 = c0 + c1 * tAc + c2 * tBc + c3 * tAc * tBc
        p2.append((c3 * sA * sB / s_req[l], (D0 - t_req[l]) / s_req[l]))
    return l0ab, p2


def _coef_table(ws):
    l0ab, p2 = _build_tables(ws)
    qa1, qa2, qb1, qb2 = l0ab
    coef = np.zeros((128, _NCOL), dtype=np.float64)
    for op in _OPS:
        k = op['kind']
        if k == 'ts_side':
            s = _node_of(0, op['t'])
            q1, q2 = (qa1, qa2) if op['side'] == 0 else (qb1, qb2)
            coef[:, op['col']] = q1[s, _KERN]
            coef[:, op['col'] + 1] = q2[s, _KERN]
        elif k == 'fix':
            n = _node_of(op['l'], op['key'])
            al, be = p2[op['l']]
            coef[:, op['col']] = al[n, _KERN]
            coef[:, op['col'] + 1] = be[n, _KERN]
        elif k == 'l6':
            al, be = p2[6]
            coef[0:64, op['col']] = al[0, _KERN[:64]]
            coef[0:64, op['col'] + 1] = be[0, _KERN[:64]]
    return coef.astype(np.float32)


def _gidx_table(idx_a, idx_b):
    """int16 gather-row indices [128, NG*NIDX//16].
    Gather g fetches tile-sides [A(2g), B(2g), A(2g+1), B(2g+1)]; row
    i = j*128 + p lands at table[i%16, 32g + i//16]."""
    gidx = np.zeros((128, NG * NIDX // 16), dtype=np.int64)
    for g in range(NG):
        for j in range(GB):
            t, side = 2 * g + j // 2, j % 2
            idx = idx_a if side == 0 else idx_b
            s = _node_of(0, t)
            ha = idx[_KERN, 0, s, 0].astype(np.int64)
            wa = idx[_KERN, 0, s, 1].astype(np.int64)
            ca = idx[_KERN, 0, s, 2].astype(np.int64)
            val = (ha * 3 + wa) * C + ca
            i = j * 128 + _LANES
            gidx[i % 16, 32 * g + i // 16] = val
    assert gidx.max() < ROWS
    return gidx.astype(np.int16)


def _xsh_core(x, core):
    """[ROWS, RSTEP] fp16: row d*64+c = image of channel c shifted by
    d=(dh*3+dw), b-interleaved over this core's two batches."""
    xs = x[B2 * core:B2 * core + B2].transpose(1, 2, 3, 0)  # [C,H,W,B2]
    flat = np.zeros(C * H * W * B2 + RSTEP, dtype=np.float32)
    flat[:C * H * W * B2] = xs.reshape(-1)
    xsh = np.empty((ROWS, RSTEP), dtype=np.float16)
    for dh in range(3):
        for dw in range(3):
            d = dh * 3 + dw
            off = (dh * W + dw) * B2
            for c in range(C):
                base = c * (H * W * B2) + off
                xsh[d * C + c] = flat[base:base + RSTEP]
    return xsh


# ---------------------------------------------------------------------------
# numpy emulator of the exact device schedule (validation aid)
# ---------------------------------------------------------------------------
def _emulate_core(xsh, gidx, coef):
    def f16(v):
        return v.astype(np.float16).astype(np.float32)
    xr = xsh.astype(np.float32)
    ab = {}
    tiles = {}
    tmp = {}
    out = np.zeros((64, 1800), dtype=np.float32)
    for op in _OPS:
        k = op['kind']
        if k == 'gather':
            g = op['g']
            cols = gidx[:, 32 * g:32 * g + 32]
            flat = cols[:16, :].T.reshape(-1)          # i -> idx
            dst = np.empty((128, GB, FW), np.float32)
            for i, idx in enumerate(flat):
                dst[i % 128, i // 128] = xr[idx, :FW]
            ab[g] = dst
        elif k == 'ts_side':
            t, side = op['t'], op['side']
            j = 2 * (t % 2) + side
            a = ab[t // 2][:, j, :]
            c = op['col']
            tmp[side] = f16(f16(a) * coef[:, c, None] + coef[:, c + 1, None])
        elif k == 'tt0':
            tmp[0] = f16(tmp[0] * tmp[1])
        elif k == 'tt':
            l, key = op['l'], op['key']
            A = tiles[(l - 1, 2 * key)]
            Bt = tiles[(l - 1, 2 * key + 1)]
            if l == 6:
                tmp['w6'] = f16(A[0:64] * Bt[64:128])
            else:
                tmp[0] = f16(A * Bt)
        elif k == 'fix':
            c = op['col']
            tiles[(op['l'], op['key'])] = f16(
                tmp[0] * coef[:, c, None] + coef[:, c + 1, None])
        elif k == 'l6':
            T5 = tiles[(5, 0)]
            w6 = f16(T5[0:64] * T5[64:128])
            c = op['col']
            o = w6 * coef[0:64, c, None] + coef[0:64, c + 1, None]
            out[:] = o.reshape(64, 30, 32, 2)[:, :, :PW, :].reshape(64, 1800)
    return out


# ---------------------------------------------------------------------------
# Bass program
# ---------------------------------------------------------------------------
_BASS_CACHE = {}


def _build_bass(debug=False):
    ck = ('nc', debug)
    if ck in _BASS_CACHE:
        return _BASS_CACHE[ck]
    import concourse.bass as bass  # noqa: F401
    import concourse.mybir as mybir
    import concourse.tile as tile
    import concourse.bacc as bacc

    f32 = mybir.dt.float32
    f16 = mybir.dt.float16
    i16 = mybir.dt.int16
    AL = mybir.AluOpType
    ACTF = mybir.ActivationFunctionType

    nc = bacc.Bacc("TRN2", target_bir_lowering=False, debug=debug,
                   num_devices=NCORES, num_swdge_queues=2)
    xsh_d = nc.dram_tensor("xsh", [ROWS, RSTEP], f16, kind="ExternalInput").ap()
    gidx_d = nc.dram_tensor("gidx", [128, NG * NIDX // 16], i16,
                            kind="ExternalInput").ap()
    coef_d = nc.dram_tensor("coef", [128, _NCOL], f32,
                            kind="ExternalInput").ap()
    out_d = nc.dram_tensor("out", [64, 1800], f32, kind="ExternalOutput").ap()

    with tile.TileContext(nc) as tc:
        with (
            tc.tile_pool(name="const", bufs=1) as pc,
            tc.tile_pool(name="gath", bufs=4) as pg,
            tc.tile_pool(name="tmp", bufs=3) as ptmp,
            tc.tile_pool(name="t0", bufs=3) as pt0,
            tc.tile_pool(name="lvl", bufs=2) as plv,
            tc.tile_pool(name="fin", bufs=1) as pfin,
            tc.tile_pool(name="outp", bufs=2) as pout,
        ):
            gidx_t = pc.tile([128, NG * NIDX // 16], i16, tag="gidx",
                             name="gidx_t")
            nc.sync.dma_start(gidx_t[:], gidx_d[:])
            coef_t = pc.tile([128, _NCOL], f32, tag="coef", name="coef_t")
            nc.sync.dma_start(coef_t[:], coef_d[:])
            warm_t = pc.tile([1, 8], f32, tag="warm", name="warm_t")
            nc.scalar.activation(warm_t[:], coef_t[0:1, 0:8],
                                 ACTF.Identity, bias=0.0, scale=1.0)

            eng = {'v': nc.vector, 'a': nc.scalar, 'p': nc.gpsimd}

            def ts(e, out_ap, in_ap, col, rows=slice(0, 128)):
                s1 = coef_t[rows, col:col + 1]
                s2 = coef_t[rows, col + 1:col + 2]
                if e == 'a':
                    nc.scalar.activation(out_ap, in_ap, ACTF.Identity,
                                         bias=s2, scale=s1)
                else:
                    eng[e].tensor_scalar(out=out_ap, in0=in_ap, scalar1=s1,
                                         scalar2=s2, op0=AL.mult, op1=AL.add)

            ab = {}
            tmp = {}
            tiles = {}
            xsh_view = xsh_d[:, 0:FW]
            for op in _OPS:
                k = op['kind']
                if k == 'gather':
                    g = op['g']
                    t_ab = pg.tile([128, GB * FW], f16, tag="AB", name="ab")
                    ab[g] = t_ab
                    nc.gpsimd.dma_gather(
                        out_ap=t_ab[:].rearrange("p (j e) -> p j e",
                                                 j=GB, e=FW),
                        in_ap=xsh_view,
                        idxs_ap=gidx_t[:, 32 * g:32 * g + 32],
                        num_idxs=NIDX,
                        num_idxs_reg=NIDX,
                        elem_size=FW,
                        elem_step=RSTEP,
                        queue_num=g % 2,
                    )
                elif k == 'ts_side':
                    t, side = op['t'], op['side']
                    j = 2 * (t % 2) + side
                    src = ab[t // 2][:, j * FW:(j + 1) * FW]
                    dst = ptmp.tile([128, FW], f16, tag="ab"[side],
                                    name="ab"[side])
                    tmp[side] = dst
                    ts(op['eng'], dst[:], src, op['col'])
                elif k == 'tt0':
                    eng[op['eng']].tensor_tensor(
                        tmp[0][:], tmp[1][:], tmp[0][:], AL.mult)
                elif k == 'tt':
                    l, key = op['l'], op['key']
                    tA = tiles[(l - 1, 2 * key)]
                    tB = tiles[(l - 1, 2 * key + 1)]
                    eng[op['eng']].tensor_tensor(
                        tA[:], tB[:], tA[:], AL.mult)
                    tmp['w'] = tA
                elif k == 'fix':
                    l, key = op['l'], op['key']
                    src = tmp[0] if l == 0 else tmp['w']
                    pool = pt0 if l == 0 else (pfin if l == 5 else plv)
                    dst = pool.tile([128, FW], f16, tag=f"T{l}",
                                    name=f"t{l}_{key}")
                    tiles[(l, key)] = dst
                    ts(op['eng'], dst[:], src[:], op['col'])
                elif k == 'l6':
                    T5 = tiles[(5, 0)]
                    t5b = pfin.tile([64, FW], f16, tag="T5b", name="t5b")
                    nc.sync.dma_start(t5b[:], T5[64:128, :])
                    nc.vector.tensor_tensor(
                        t5b[:], T5[0:64, :], t5b[:], AL.mult)
                    w6v = t5b[:].rearrange("p (h w b) -> p h w b",
                                           h=30, w=32, b=2)[:, :, 0:PW, :]
                    out_t = pout.tile([64, 1800], f32, tag="out", name="outt")
                    c = op['col']
                    for hh in (0, 1):
                        ov = out_t[:, 900 * hh:900 * hh + 900].rearrange(
                            "p (h w b) -> p h w b", h=15, w=PW, b=2)
                        nc.scalar.activation(
                            ov, w6v[:, 15 * hh:15 * hh + 15, :, :],
                            ACTF.Identity,
                            bias=coef_t[0:64, c + 1:c + 2],
                            scale=coef_t[0:64, c:c + 1])
                        nc.sync.dma_start(
                            out_d[:, 900 * hh:900 * hh + 900],
                            out_t[:, 900 * hh:900 * hh + 900])
    nc.compile()
    _BASS_CACHE[ck] = nc
    return nc


# ---------------------------------------------------------------------------
# entry points
# ---------------------------------------------------------------------------
def _prep_inputs(x, idx_a, idx_b, ws):
    coef = _coef_table(ws)
    gidx = _gidx_table(idx_a, idx_b)
    x = np.ascontiguousarray(x, dtype=np.float32)
    return [{"xsh": _xsh_core(x, core), "gidx": gidx, "coef": coef}
            for core in range(NCORES)]


def _assemble(core_outs):
    full = np.stack(core_outs).astype(np.float32)   # [8, 64, 1800]
    full = full.reshape(NCORES, K, P, B2)
    full = full.transpose(0, 3, 1, 2).reshape(B, K, P, 1)
    return np.ascontiguousarray(full)


def kernel(x, idx_a, idx_b, w0, w1, w2, w3, w4, w5, w6):
    ws = [np.asarray(w, dtype=np.float32) for w in
          (w0, w1, w2, w3, w4, w5, w6)]
    x = np.asarray(x, dtype=np.float32)
    idx_a = np.asarray(idx_a, dtype=np.int32)
    idx_b = np.asarray(idx_b, dtype=np.int32)
    in_maps = _prep_inputs(x, idx_a, idx_b, ws)
    nc = _build_bass()
    from concourse.bass_utils import run_bass_kernel_spmd
    res = run_bass_kernel_spmd(nc, in_maps, core_ids=list(range(NCORES)))
    return _assemble([r["out"] for r in res.results])


def kernel_emulate(x, idx_a, idx_b, w0, w1, w2, w3, w4, w5, w6):
    """Pure-numpy emulation of the exact device schedule (debug aid)."""
    ws = [np.asarray(w, dtype=np.float32) for w in
          (w0, w1, w2, w3, w4, w5, w6)]
    in_maps = _prep_inputs(np.asarray(x, np.float32),
                           np.asarray(idx_a, np.int32),
                           np.asarray(idx_b, np.int32), ws)
    outs = [_emulate_core(m["xsh"], m["gidx"].astype(np.int64), m["coef"])
            for m in in_maps]
    return _assemble(outs)


# revision 6
# speedup vs baseline: 1.5460x; 1.1681x over previous
"""Trainium2 Bass kernel for nn_LogicConv3d (differentiable-logic conv tree).

Problem (hardcoded): x [16,64,32,32] f32; idx_a/idx_b [64,900,64,3] i32;
w0..w6 [s,64,16] f32 (s = 64,32,16,8,4,2,1). Output [16,64,900,1] f32.

v3 design (fp16 product-form tree, 4-batch x 32-kernel sharding):
 - Sharding: core c handles batches [4*(c%4) .. +4) and kernels
   [32*(c//4) .. +32).  Wider rows (4 batches interleaved) halve the
   gather-descriptor count and per-op overheads vs batch-only sharding.
 - Algebra: every stored node value is an affine image X = (V - t)/s of the
   true node value V in [0,1].  Choosing the children's t as -C2/C3, -C1/C3
   makes each tree node an exact PRODUCT of its children's stored values:
   1 tensor_tensor(mult) + 1 tensor_scalar (affine fix) per node -- both
   have DVE fast modes in fp16, unlike scalar_tensor_tensor (none).
   L0 pre-shifts the raw leaves (2 extra ts).  All constants fold into the
   scalars; rel err ~7e-4 vs the 2e-2 gate (validated in emulation and on HW).
 - Gather: the x-slice is laid out in HBM as 9 shifted copies
   ([576, 4096] fp16 rows: (dh*3+dw)*64+c = channel-c image shifted by
   (dh,dw), 4 batches interleaved) so leaf crops are aligned rows fetched
   by batched dma_gather (int16 row ids, 256 rows / instruction).
 - Lanes: (group g = lane>>5, kernel k = lane&31); tree level l<=4 keeps
   groups independent; L5/L6 cross groups via SBUF->SBUF DMA realigns.
 - Compute is "wide" (3840 = 30h x 32w x 4b free elems, garbage at w=30,31)
   so all operands stay packed (DVE 4x/2x eligible); the final activation
   compacts to 900 windows x 4 batches in fp32.
"""
import numpy as np

B, C, H, W = 16, 64, 32, 32
K = 64
DEPTH = 6
PW = 30
P = PW * PW        # 900
NCORES = 8
B4 = 4             # batches per core
KPC = 32           # kernels per core
GRP = 4            # node groups in the lane dim
FW = 30 * 32 * B4  # 3840 wide free elems per lane
RSTEP = H * W * B4 # 4096: gather row stride (elements)
NSH = 9
ROWS = NSH * C     # 576 gather source rows
NT0 = 16           # L0 tiles
GB = 2             # tile-sides per dma_gather instruction
NG = 2 * NT0 // GB # 16 gather instructions
NIDX = GB * 128    # 256 idxs per gather

GATE_M = np.array([
    [0, 0, 0, 0], [0, 0, 0, 1], [0, 1, 0, -1], [0, 1, 0, 0],
    [0, 0, 1, -1], [0, 0, 1, 0], [0, 1, 1, -2], [0, 1, 1, -1],
    [1, -1, -1, 1], [1, -1, -1, 2], [1, 0, -1, 0], [1, 0, -1, 1],
    [1, -1, 0, 0], [1, -1, 0, 1], [1, 0, 0, -1], [1, 0, 0, 0],
], dtype=np.float64)


def _softmax64(w):
    w = np.asarray(w, np.float64)
    e = np.exp(w - w.max(-1, keepdims=True))
    return e / e.sum(-1, keepdims=True)


# ---------------------------------------------------------------------------
# static op schedule with greedy engine assignment
# ---------------------------------------------------------------------------
def _schedule():
    """DFS op list.  Engines: 'v' DVE, 'a' Act ('p' Pool is reserved for
    gather dispatch).  Costs are measured per-op us on [*, 3840] fp16."""
    ops = []
    col = [0]
    loads = {'v': 0.0, 'a': 0.0}
    TS = {'v': 1.28, 'a': 3.60}
    TT = {'v': 2.25}
    ngather = [0]

    def pick(cost):
        e = min(cost, key=lambda k: loads[k] + cost[k])
        loads[e] += cost[e]
        return e

    def need_gather(upto):
        while ngather[0] <= min(upto, NG - 1):
            ops.append({'kind': 'gather', 'g': ngather[0]})
            ngather[0] += 1

    def alloc_col():
        c = col[0]
        col[0] += 2
        return c

    def emit(l, key):
        if l == 0:
            t = key
            need_gather(t + 2)
            ops.append({'kind': 'ts_side', 'side': 0, 't': t,
                        'col': alloc_col(), 'eng': pick(TS)})
            ops.append({'kind': 'ts_side', 'side': 1, 't': t,
                        'col': alloc_col(), 'eng': pick(TS)})
            ops.append({'kind': 'tt0', 't': t, 'eng': pick(TT)})
            ops.append({'kind': 'fix', 'l': 0, 'key': t,
                        'col': alloc_col(), 'eng': pick(TS)})
            return
        emit(l - 1, 2 * key)
        emit(l - 1, 2 * key + 1)
        ops.append({'kind': 'tt', 'l': l, 'key': key, 'eng': pick(TT)})
        ops.append({'kind': 'fix', 'l': l, 'key': key,
                    'col': alloc_col(), 'eng': pick(TS)})
    emit(4, 0)
    # L5: node n merges groups (2n, 2n+1) of T4; realign to base-0 first.
    ops.append({'kind': 'l5', 'n': 0, 'col': alloc_col(), 'eng': pick(TS)})
    ops.append({'kind': 'l5', 'n': 1, 'col': alloc_col(), 'eng': pick(TS)})
    ops.append({'kind': 'l6', 'col': alloc_col()})
    return ops, col[0]


_OPS, _NCOL = _schedule()

_LANES = np.arange(128)
_G = _LANES >> 5          # group 0..3
_KL = _LANES & 31         # kernel-in-core 0..31


def _node_of(l, key):
    """Tree-node index per lane for a level-l tile (l <= 4)."""
    if l == 0:
        return key + NT0 * _G
    return (_G << (4 - l)) + key


# ---------------------------------------------------------------------------
# host tables
# ---------------------------------------------------------------------------
def _build_tables(ws):
    """Per-(node,kern) scalars over the FULL K=64, f64."""
    cs = [np.einsum('skg,gj->skj', _softmax64(w), GATE_M) for w in ws]
    s_req = [np.ones((2 ** (DEPTH - l), K)) for l in range(DEPTH + 1)]
    t_req = [np.zeros((2 ** (DEPTH - l), K)) for l in range(DEPTH + 1)]
    for l in range(DEPTH, 0, -1):
        c = cs[l]
        c1, c2, c3 = c[..., 1], c[..., 2], c[..., 3]
        tA, tB = -c2 / c3, -c1 / c3
        t_req[l - 1][0::2], t_req[l - 1][1::2] = tA, tB
        s_req[l - 1][0::2] = (1 + np.abs(tA)) / 2
        s_req[l - 1][1::2] = (1 + np.abs(tB)) / 2
    c = cs[0]
    c1, c2, c3 = c[..., 1], c[..., 2], c[..., 3]
    ta, tb = -c2 / c3, -c1 / c3
    sa, sb = (1 + np.abs(ta)) / 2, (1 + np.abs(tb)) / 2
    l0ab = (1 / sa, -ta / sa, 1 / sb, -tb / sb)
    p2 = []
    for l in range(DEPTH + 1):
        c = cs[l]
        c0, c1, c2, c3 = c[..., 0], c[..., 1], c[..., 2], c[..., 3]
        if l == 0:
            tAc, tBc, sA, sB = ta, tb, sa, sb
        else:
            tAc, tBc = t_req[l - 1][0::2], t_req[l - 1][1::2]
            sA, sB = s_req[l - 1][0::2], s_req[l - 1][1::2]
        D0 = c0 + c1 * tAc + c2 * tBc + c3 * tAc * tBc
        p2.append((c3 * sA * sB / s_req[l], (D0 - t_req[l]) / s_req[l]))
    return l0ab, p2


def _coef_table(ws, kg):
    """[128, _NCOL] f32 for kernel-group kg (kernels 32kg..32kg+31)."""
    l0ab, p2 = _build_tables(ws)
    qa1, qa2, qb1, qb2 = l0ab
    kern = KPC * kg + _KL
    coef = np.zeros((128, _NCOL), dtype=np.float64)
    for op in _OPS:
        k = op['kind']
        if k == 'ts_side':
            s = _node_of(0, op['t'])
            q1, q2 = (qa1, qa2) if op['side'] == 0 else (qb1, qb2)
            coef[:, op['col']] = q1[s, kern]
            coef[:, op['col'] + 1] = q2[s, kern]
        elif k == 'fix':
            n = _node_of(op['l'], op['key'])
            al, be = p2[op['l']]
            coef[:, op['col']] = al[n, kern]
            coef[:, op['col'] + 1] = be[n, kern]
        elif k == 'l5':
            al, be = p2[5]
            coef[0:32, op['col']] = al[op['n'], kern[0:32]]
            coef[0:32, op['col'] + 1] = be[op['n'], kern[0:32]]
        elif k == 'l6':
            al, be = p2[6]
            coef[0:32, op['col']] = al[0, kern[0:32]]
            coef[0:32, op['col'] + 1] = be[0, kern[0:32]]
    return coef.astype(np.float32)


def _gidx_table(idx_a, idx_b, kg):
    """int16 gather-row indices [128, NG*NIDX//16] for kernel-group kg.
    Gather g fetches tile-sides [A(g&~1|..)]: sides list order is
    (t=GB*g//2 + j//2, side=j%2); row i = j*128 + p lands at
    table[i%16, (NIDX//16)*g + i//16]."""
    ncols = NIDX // 16
    gidx = np.zeros((128, NG * ncols), dtype=np.int64)
    kern = KPC * kg + _KL
    for g in range(NG):
        for j in range(GB):
            t, side = (GB * g + j) // 2, (GB * g + j) % 2
            idx = idx_a if side == 0 else idx_b
            s = _node_of(0, t)
            ha = idx[kern, 0, s, 0].astype(np.int64)
            wa = idx[kern, 0, s, 1].astype(np.int64)
            ca = idx[kern, 0, s, 2].astype(np.int64)
            val = (ha * 3 + wa) * C + ca
            i = j * 128 + _LANES
            gidx[i % 16, ncols * g + i // 16] = val
    assert gidx.max() < ROWS
    return gidx.astype(np.int16)


def _xsh_core(x, bg):
    """[ROWS, RSTEP] fp16 for batch-group bg: row d*64+c = channel-c image
    shifted by d=(dh*3+dw), batches interleaved innermost."""
    xs = x[B4 * bg:B4 * bg + B4].transpose(1, 2, 3, 0)  # [C,H,W,B4]
    flat = np.zeros(C * H * W * B4 + RSTEP, dtype=np.float32)
    flat[:C * H * W * B4] = xs.reshape(-1)
    xsh = np.empty((ROWS, RSTEP), dtype=np.float16)
    for dh in range(3):
        for dw in range(3):
            d = dh * 3 + dw
            off = (dh * W + dw) * B4
            for c in range(C):
                base = c * (H * W * B4) + off
                xsh[d * C + c] = flat[base:base + RSTEP]
    return xsh


# ---------------------------------------------------------------------------
# numpy emulator of the exact device schedule (validation aid)
# ---------------------------------------------------------------------------
def _emulate_core(xsh, gidx, coef):
    def f16(v):
        return v.astype(np.float16).astype(np.float32)
    xr = xsh.astype(np.float32)
    ab = {}
    tiles = {}
    tmp = {}
    x5 = {}
    ncols = NIDX // 16
    out = np.zeros((KPC, P * B4), dtype=np.float32)
    for op in _OPS:
        k = op['kind']
        if k == 'gather':
            g = op['g']
            cols = gidx[:, ncols * g:ncols * (g + 1)]
            flat = cols[:16, :].T.reshape(-1)
            dst = np.empty((128, GB, FW), np.float32)
            for i, idx in enumerate(flat):
                dst[i % 128, i // 128] = xr[idx, :FW]
            ab[g] = dst
        elif k == 'ts_side':
            t, side = op['t'], op['side']
            gi, j = (2 * t + side) // GB, (2 * t + side) % GB
            a = ab[gi][:, j, :]
            c = op['col']
            tmp[side] = f16(f16(a) * coef[:, c, None] + coef[:, c + 1, None])
        elif k == 'tt0':
            tmp[0] = f16(tmp[0] * tmp[1])
        elif k == 'tt':
            l, key = op['l'], op['key']
            tmp[0] = f16(tiles[(l - 1, 2 * key)] * tiles[(l - 1, 2 * key + 1)])
        elif k == 'fix':
            c = op['col']
            tiles[(op['l'], op['key'])] = f16(
                tmp[0] * coef[:, c, None] + coef[:, c + 1, None])
        elif k == 'l5':
            n, c = op['n'], op['col']
            T4 = tiles[(4, 0)]
            w = f16(T4[64 * n:64 * n + 32] * T4[64 * n + 32:64 * n + 64])
            x5[n] = f16(w * coef[0:32, c, None] + coef[0:32, c + 1, None])
        elif k == 'l6':
            c = op['col']
            w = f16(x5[0] * x5[1])
            o = w * coef[0:32, c, None] + coef[0:32, c + 1, None]
            out[:] = o.reshape(KPC, 30, 32, B4)[:, :, :PW, :].reshape(
                KPC, P * B4)
    return out


# ---------------------------------------------------------------------------
# Bass program
# ---------------------------------------------------------------------------
_BASS_CACHE = {}


def _build_bass(debug=False):
    ck = ('nc', debug)
    if ck in _BASS_CACHE:
        return _BASS_CACHE[ck]
    import concourse.bass as bass  # noqa: F401
    import concourse.mybir as mybir
    import concourse.tile as tile
    import concourse.bacc as bacc

    f32 = mybir.dt.float32
    f16 = mybir.dt.float16
    i16 = mybir.dt.int16
    AL = mybir.AluOpType
    ACTF = mybir.ActivationFunctionType
    ncols = NIDX // 16

    nc = bacc.Bacc("TRN2", target_bir_lowering=False, debug=debug,
                   num_devices=NCORES, num_swdge_queues=2)
    xsh_d = nc.dram_tensor("xsh", [ROWS, RSTEP], f16, kind="ExternalInput").ap()
    gidx_d = nc.dram_tensor("gidx", [128, NG * ncols], i16,
                            kind="ExternalInput").ap()
    coef_d = nc.dram_tensor("coef", [128, _NCOL], f32,
                            kind="ExternalInput").ap()
    out_d = nc.dram_tensor("out", [KPC, P * B4], f32,
                           kind="ExternalOutput").ap()

    with tile.TileContext(nc) as tc:
        with (
            tc.tile_pool(name="const", bufs=1) as pc,
            tc.tile_pool(name="gath", bufs=3) as pg,
            tc.tile_pool(name="tmp", bufs=2) as ptmp,
            tc.tile_pool(name="t0", bufs=2) as pt0,
            tc.tile_pool(name="lvl", bufs=2) as plv,
            tc.tile_pool(name="fin", bufs=1) as pfin,
            tc.tile_pool(name="outp", bufs=1) as pout,
        ):
            gidx_t = pc.tile([128, NG * ncols], i16, tag="gidx",
                             name="gidx_t")
            nc.sync.dma_start(gidx_t[:], gidx_d[:])
            coef_t = pc.tile([128, _NCOL], f32, tag="coef", name="coef_t")
            nc.sync.dma_start(coef_t[:], coef_d[:])
            warm_t = pc.tile([1, 8], f32, tag="warm", name="warm_t")
            nc.scalar.activation(warm_t[:], coef_t[0:1, 0:8],
                                 ACTF.Identity, bias=0.0, scale=1.0)

            eng = {'v': nc.vector, 'a': nc.scalar}

            def ts(e, out_ap, in_ap, col, rows=slice(0, 128)):
                s1 = coef_t[rows, col:col + 1]
                s2 = coef_t[rows, col + 1:col + 2]
                if e == 'a':
                    nc.scalar.activation(out_ap, in_ap, ACTF.Identity,
                                         bias=s2, scale=s1)
                else:
                    eng[e].tensor_scalar(out=out_ap, in0=in_ap, scalar1=s1,
                                         scalar2=s2, op0=AL.mult, op1=AL.add)

            ab = {}
            tmp = {}
            tiles = {}
            x5 = {}
            xsh_view = xsh_d[:, 0:FW]
            for op in _OPS:
                k = op['kind']
                if k == 'gather':
                    g = op['g']
                    t_ab = pg.tile([128, GB * FW], f16, tag="AB", name="ab")
                    ab[g] = t_ab
                    nc.gpsimd.dma_gather(
                        out_ap=t_ab[:].rearrange("p (j e) -> p j e",
                                                 j=GB, e=FW),
                        in_ap=xsh_view,
                        idxs_ap=gidx_t[:, ncols * g:ncols * (g + 1)],
                        num_idxs=NIDX,
                        num_idxs_reg=NIDX,
                        elem_size=FW,
                        elem_step=RSTEP,
                        queue_num=g % 2,
                    )
                elif k == 'ts_side':
                    t, side = op['t'], op['side']
                    gi, j = (2 * t + side) // GB, (2 * t + side) % GB
                    src = ab[gi][:, j * FW:(j + 1) * FW]
                    dst = ptmp.tile([128, FW], f16, tag="ab"[side],
                                    name="ab"[side])
                    tmp[side] = dst
                    ts(op['eng'], dst[:], src, op['col'])
                elif k == 'tt0':
                    eng[op['eng']].tensor_tensor(
                        tmp[0][:], tmp[1][:], tmp[0][:], AL.mult)
                elif k == 'tt':
                    l, key = op['l'], op['key']
                    tA = tiles[(l - 1, 2 * key)]
                    tB = tiles[(l - 1, 2 * key + 1)]
                    eng[op['eng']].tensor_tensor(
                        tA[:], tB[:], tA[:], AL.mult)
                    tmp['w'] = tA
                elif k == 'fix':
                    l, key = op['l'], op['key']
                    src = tmp[0] if l == 0 else tmp['w']
                    pool = pt0 if l == 0 else (pfin if l == 4 else plv)
                    dst = pool.tile([128, FW], f16, tag=f"T{l}",
                                    name=f"t{l}_{key}")
                    tiles[(l, key)] = dst
                    ts(op['eng'], dst[:], src[:], op['col'])
                elif k == 'l5':
                    # node n merges T4 groups (2n, 2n+1); realign slabs to
                    # partition base 0 (multi-operand ops need equal bases)
                    n, c = op['n'], op['col']
                    T4 = tiles[(4, 0)]
                    ra = pfin.tile([32, FW], f16, tag=f"r{n}", name=f"r{n}")
                    nc.sync.dma_start(ra[:], T4[64 * n + 32:64 * n + 64, :])
                    if n == 0:
                        left = T4[0:32, :]
                    else:
                        rl = pfin.tile([32, FW], f16, tag="rl", name="rl")
                        nc.sync.dma_start(rl[:], T4[64:96, :])
                        left = rl[:]
                    nc.vector.tensor_tensor(ra[:], left, ra[:], AL.mult)
                    dst = pfin.tile([32, FW], f16, tag=f"X5{n}",
                                    name=f"x5{n}")
                    x5[n] = dst
                    ts(op['eng'], dst[:], ra[:], c, rows=slice(0, 32))
                elif k == 'l6':
                    c = op['col']
                    nc.vector.tensor_tensor(
                        x5[0][:], x5[1][:], x5[0][:], AL.mult)
                    w6v = x5[0][:].rearrange("p (h w b) -> p h w b",
                                             h=30, w=32, b=B4)[:, :, 0:PW, :]
                    half = P * B4 // 2
                    for hh in (0, 1):
                        out_t = pout.tile([KPC, half], f32, tag=f"out{hh}",
                                          name=f"outt{hh}")
                        ov = out_t[:].rearrange(
                            "p (h w b) -> p h w b", h=15, w=PW, b=B4)
                        nc.scalar.activation(
                            ov, w6v[:, 15 * hh:15 * hh + 15, :, :],
                            ACTF.Identity,
                            bias=coef_t[0:KPC, c + 1:c + 2],
                            scale=coef_t[0:KPC, c:c + 1])
                        nc.sync.dma_start(
                            out_d[:, half * hh:half * (hh + 1)], out_t[:])
    nc.compile()
    _BASS_CACHE[ck] = nc
    return nc


# ---------------------------------------------------------------------------
# entry points
# ---------------------------------------------------------------------------
def _prep_inputs(x, idx_a, idx_b, ws):
    x = np.ascontiguousarray(x, dtype=np.float32)
    coefs = [_coef_table(ws, kg) for kg in range(2)]
    gidxs = [_gidx_table(idx_a, idx_b, kg) for kg in range(2)]
    xshs = [_xsh_core(x, bg) for bg in range(4)]
    in_maps = []
    for core in range(NCORES):
        bg, kg = core % 4, core // 4
        in_maps.append({"xsh": xshs[bg], "gidx": gidxs[kg],
                        "coef": coefs[kg]})
    return in_maps


def _assemble(core_outs):
    full = np.zeros((B, K, P, 1), dtype=np.float32)
    for core, o in enumerate(core_outs):
        bg, kg = core % 4, core // 4
        o = np.asarray(o, np.float32).reshape(KPC, P, B4)
        full[B4 * bg:B4 * bg + B4, KPC * kg:KPC * kg + KPC, :, 0] = \
            o.transpose(2, 0, 1)
    return full


def kernel(x, idx_a, idx_b, w0, w1, w2, w3, w4, w5, w6):
    ws = [np.asarray(w, dtype=np.float32) for w in
          (w0, w1, w2, w3, w4, w5, w6)]
    x = np.asarray(x, dtype=np.float32)
    idx_a = np.asarray(idx_a, dtype=np.int32)
    idx_b = np.asarray(idx_b, dtype=np.int32)
    in_maps = _prep_inputs(x, idx_a, idx_b, ws)
    nc = _build_bass()
    from concourse.bass_utils import run_bass_kernel_spmd
    res = run_bass_kernel_spmd(nc, in_maps, core_ids=list(range(NCORES)))
    return _assemble([r["out"] for r in res.results])


def kernel_emulate(x, idx_a, idx_b, w0, w1, w2, w3, w4, w5, w6):
    """Pure-numpy emulation of the exact device schedule (debug aid)."""
    ws = [np.asarray(w, dtype=np.float32) for w in
          (w0, w1, w2, w3, w4, w5, w6)]
    in_maps = _prep_inputs(np.asarray(x, np.float32),
                           np.asarray(idx_a, np.int32),
                           np.asarray(idx_b, np.int32), ws)
    outs = [_emulate_core(m["xsh"], m["gidx"].astype(np.int64), m["coef"])
            for m in in_maps]
    return _assemble(outs)
